# revision 5
# baseline (speedup 1.0000x reference)
"""Trainium2 Bass kernel for nn_CausalFullAttention_37821482009327.

Causal full attention (no softmax) with data-dependent complex relative
position decay, silu gating, and output projection.

Sharding: tensor-parallel over the 16 heads -> 2 heads per NeuronCore x 8.
Each core computes its heads' attention and a partial out-projection
(contraction over its 128-wide dim_inner slice); the host sums the 8
partials (the "all-reduce" happens at gather time).

Shapes (hardcoded): B=1, N=2048, D=1024, H=16, Dh=64, Dc=32.
"""
import sys

sys.path.insert(0, "/opt/trn_rl_repo")

import numpy as np

import concourse.bass as bass
import concourse.tile as tile
from concourse import bacc, mybir
from concourse.bass_utils import run_bass_kernel_spmd
from concourse.masks import make_identity

F32 = mybir.dt.float32
F32R = mybir.dt.float32r  # TF32-class matmul fast path (1 cyc/row vs 4)

N = 2048
D = 1024
H_LOC = 2          # heads per core
DH = 64
DC = 32
NCORES = 8
EPS = 1e-10

NCH = N // 128     # 16 n-chunks of 128
DCH = D // 128     # 8 d-chunks of 128
NC4 = N // 512     # 4 n-chunks of 512


def _emit(nc):
    """Emit the per-core program (SPMD: same program, per-core weight data)."""
    X = nc.dram_tensor("X", [N, D], F32, kind="ExternalInput")
    WQ = nc.dram_tensor("WQ", [D, 128], F32R, kind="ExternalInput")
    WK = nc.dram_tensor("WK", [D, 128], F32R, kind="ExternalInput")
    WAV = nc.dram_tensor("WAV", [D, 256], F32R, kind="ExternalInput")
    WG = nc.dram_tensor("WG", [D, 128], F32R, kind="ExternalInput")
    WO = nc.dram_tensor("WO", [128, D], F32R, kind="ExternalInput")
    BT = nc.dram_tensor("BT", [128, 8], F32, kind="ExternalInput")
    OUT = nc.dram_tensor("OUT", [D, N], F32, kind="ExternalOutput")

    with tile.TileContext(nc) as tc:
        with tc.tile_pool(name="pers", bufs=1) as pers:
            # ---- persistent SBUF tensors ----
            ident = pers.tile([128, 128], F32, tag="ident")
            make_identity(nc, ident[:])
            maskt = pers.tile([128, 4 * 512], F32, tag="maskt")
            for off in range(4):
                m = maskt[:, off * 512:(off + 1) * 512]
                nc.gpsimd.memset(m, 1.0)
                # keep (== leave 1.0) where f >= p + 128*off, else 0
                nc.gpsimd.affine_select(
                    out=m, in_=m, compare_op=mybir.AluOpType.is_ge,
                    fill=0.0, base=-128 * off, pattern=[[1, 512]],
                    channel_multiplier=-1)
            btile = pers.tile([128, 8], F32, tag="btile")
            nc.sync.dma_start(btile[:], BT[:])

            xt = [pers.tile([128, N], F32R, tag=f"xt{dc}", name=f"xt{dc}")
                  for dc in range(DCH)]
            qt = pers.tile([128, N], F32R, tag="qt")
            kt = pers.tile([128, N], F32R, tag="kt")
            gsilu = pers.tile([128, N], F32, tag="gsilu")
            acrT = pers.tile([128, N], F32, tag="acrT")
            krT = pers.tile([128, N], F32, tag="krT")
            vb = pers.tile([128, N], F32R, tag="vb")  # v natural: [j_lo][nc*128+col]

            wq_sb = [pers.tile([128, 128], F32R, tag=f"wq{dc}", name=f"wq{dc}")
                     for dc in range(DCH)]
            wk_sb = [pers.tile([128, 128], F32R, tag=f"wk{dc}", name=f"wk{dc}")
                     for dc in range(DCH)]
            wg_sb = [pers.tile([128, 128], F32R, tag=f"wg{dc}", name=f"wg{dc}")
                     for dc in range(DCH)]
            wo_sb = [pers.tile([128, 128], F32R, tag=f"wo{ji}", name=f"wo{ji}")
                     for ji in range(DCH)]
            for dc in range(DCH):
                nc.sync.dma_start(wq_sb[dc][:], WQ[dc * 128:(dc + 1) * 128, :])
                nc.sync.dma_start(wk_sb[dc][:], WK[dc * 128:(dc + 1) * 128, :])
                nc.sync.dma_start(wg_sb[dc][:], WG[dc * 128:(dc + 1) * 128, :])
                nc.sync.dma_start(wo_sb[dc][:], WO[:, dc * 128:(dc + 1) * 128])

            # ---- phase A: x -> xT via PE transpose ----
            with (
                tc.tile_pool(name="xnat", bufs=5) as xnat,
                tc.tile_pool(name="psa", bufs=2, space="PSUM") as psa,
            ):
                for ncg in range(4):  # groups of 4 n-chunks
                    xn = []
                    for i in range(4):
                        t = xnat.tile([128, D], F32, tag="xn", name="xn")
                        nc.sync.dma_start(
                            t[:], X[(ncg * 4 + i) * 128:(ncg * 4 + i + 1) * 128, :])
                        xn.append(t)
                    for dc in range(DCH):
                        pt = psa.tile([128, 512], F32, tag="ptr")
                        for i in range(4):
                            nc.tensor.transpose(
                                pt[:, i * 128:(i + 1) * 128],
                                xn[i][:, dc * 128:(dc + 1) * 128], ident[:])
                        nc.vector.tensor_copy(
                            xt[dc][:, ncg * 512:(ncg + 1) * 512], pt[:])

            # ---- phase B+C: a,v projection; complex cumprod scan; acr ----
            with (
                tc.tile_pool(name="scan", bufs=1) as scan,
                tc.tile_pool(name="wavp", bufs=1) as wavp,
                tc.tile_pool(name="aep", bufs=2) as aep,
                tc.tile_pool(name="psb", bufs=2, space="PSUM") as psb,
            ):
                wav_sb = [wavp.tile([128, 256], F32R, tag=f"wav{dc}", name=f"wav{dc}")
                          for dc in range(DCH)]
                for dc in range(DCH):
                    nc.sync.dma_start(wav_sb[dc][:], WAV[dc * 128:(dc + 1) * 128, :])

                # scan buffers: [128, 16 blocks-of-(2h) x 64]; per 64-block:
                # [0:32]=pad, [32:64]=d data
                reA = scan.tile([128, N], F32, tag="reA")
                imA = scan.tile([128, N], F32, tag="imA")
                reB = scan.tile([128, N], F32, tag="reB")
                imB = scan.tile([128, N], F32, tag="imB")
                t1 = scan.tile([128, 1024], F32, tag="t1")
                t2 = scan.tile([128, 1024], F32, tag="t2")

                def blocks(buf, o, w=32):
                    # [p][32 blocks step 64][w cols at offset o]
                    return buf[:].rearrange("p (b w) -> p b w", w=64)[:, :, o:o + w]

                nc.gpsimd.memset(blocks(reA, 0), 1.0)
                nc.gpsimd.memset(blocks(imA, 0), 0.0)
                nc.gpsimd.memset(blocks(reB, 0), 1.0)
                nc.gpsimd.memset(blocks(imB, 0), 0.0)

                for nci in range(NCH):
                    pav = psb.tile([128, 256], F32, tag="pav")
                    for dc in range(DCH):
                        nc.tensor.matmul(
                            pav[:], xt[dc][:, nci * 128:(nci + 1) * 128],
                            wav_sb[dc][:], start=(dc == 0), stop=(dc == DCH - 1))
                    base = nci * 128
                    # a_re -> reA data cols, a_im -> imA, v -> vb (f32r)
                    dst_re = reA[:, base:base + 128].rearrange(
                        "p (h w) -> p h w", w=64)[:, :, 32:64]
                    dst_im = imA[:, base:base + 128].rearrange(
                        "p (h w) -> p h w", w=64)[:, :, 32:64]
                    src_re = pav[:, 0:64].rearrange("p (h w) -> p h w", w=32)
                    src_im = pav[:, 64:128].rearrange("p (h w) -> p h w", w=32)
                    nc.vector.tensor_copy(dst_re, src_re)
                    nc.vector.tensor_copy(dst_im, src_im)
                    nc.vector.tensor_copy(vb[:, base:base + 128], pav[:, 128:256])

                # pointwise: ac = a * sigmoid(|a|)/|a|
                t1v = t1[:].rearrange("p (b w) -> p b w", w=32)
                t2v = t2[:].rearrange("p (b w) -> p b w", w=32)
                reD = blocks(reA, 32)
                imD = blocks(imA, 32)
                nc.vector.tensor_mul(t1v, reD, reD)
                nc.vector.tensor_mul(t2v, imD, imD)
                nc.vector.tensor_add(t1v, t1v, t2v)          # |a|^2
                nc.scalar.activation(t2[:], t1[:], mybir.ActivationFunctionType.Sqrt)
                nc.vector.reciprocal_approx_fast(t1[:], t2[:])   # 1/|a|
                nc.scalar.activation(t2[:], t2[:],
                                     mybir.ActivationFunctionType.Sigmoid)
                nc.vector.tensor_mul(t1v, t1v, t2v)          # f = sig(|a|)/|a|
                nc.vector.tensor_mul(reD, reD, t1v)
                nc.vector.tensor_mul(imD, imD, t1v)

                # doubling scan: c[d] = c[d] * c[d - s] (complex), s=1,2,4,8,16
                src_re, src_im, dst_re_b, dst_im_b = reA, imA, reB, imB
                for si, s in enumerate((1, 2, 4, 8, 16)):
                    r0 = blocks(src_re, 32)
                    i0 = blocks(src_im, 32)
                    rs = blocks(src_re, 32 - s)
                    is_ = blocks(src_im, 32 - s)
                    rd = blocks(dst_re_b, 32)
                    id_ = blocks(dst_im_b, 32)
                    nc.vector.tensor_mul(t1v, r0, rs)
                    nc.vector.tensor_mul(t2v, i0, is_)
                    nc.vector.tensor_sub(rd, t1v, t2v)
                    if si < 4:
                        nc.vector.tensor_mul(t1v, r0, is_)
                        nc.vector.tensor_mul(t2v, i0, rs)
                        nc.vector.tensor_add(id_, t1v, t2v)
                    src_re, dst_re_b = dst_re_b, src_re
                    src_im, dst_im_b = dst_im_b, src_im
                # after 5 steps the final real part lives in reB
                for nci in range(NCH):
                    ae = aep.tile([128, 128], F32, tag="ae")
                    base = nci * 128
                    src = reB[:, base:base + 128].rearrange(
                        "p (h w) -> p h w", w=64)[:, :, 32:64]
                    for c in range(2):
                        # dst cols h*64 + 2d + c
                        dst = ae[:].rearrange(
                            "p (h d two) -> p h d two", h=2, two=2)[:, :, :, c]
                        nc.vector.tensor_scalar_max(dst, src, EPS)
                    pae = psb.tile([128, 128], F32, tag="pae")
                    nc.tensor.transpose(pae[:], ae[:], ident[:])
                    nc.vector.tensor_copy(acrT[:, base:base + 128], pae[:])
                nc.vector.reciprocal_approx_fast(krT[:], acrT[:])

            # ---- phase D: q/k/g projections ----
            with tc.tile_pool(name="psd", bufs=2, space="PSUM") as psd:
                for c4 in range(NC4):
                    ns = slice(c4 * 512, (c4 + 1) * 512)
                    pq = psd.tile([128, 512], F32, tag="pq")
                    for dc in range(DCH):
                        nc.tensor.matmul(pq[:], wq_sb[dc][:], xt[dc][:, ns],
                                         start=(dc == 0), stop=(dc == DCH - 1))
                    nc.vector.tensor_mul(qt[:, ns], pq[:], acrT[:, ns])
                    pk = psd.tile([128, 512], F32, tag="pk")
                    for dc in range(DCH):
                        nc.tensor.matmul(pk[:], wk_sb[dc][:], xt[dc][:, ns],
                                         start=(dc == 0), stop=(dc == DCH - 1))
                    nc.vector.tensor_mul(kt[:, ns], pk[:], krT[:, ns])
                    pg = psd.tile([128, 512], F32, tag="pg")
                    for dc in range(DCH):
                        nc.tensor.matmul(pg[:], wg_sb[dc][:], xt[dc][:, ns],
                                         start=(dc == 0), stop=(dc == DCH - 1))
                    nc.scalar.activation(gsilu[:, ns], pg[:],
                                         mybir.ActivationFunctionType.Silu)

            # ---- phase E: attention + gating + partial out-projection ----
            with (
                tc.tile_pool(name="sse", bufs=4) as sse,
                tc.tile_pool(name="gte", bufs=2) as gte,
                tc.tile_pool(name="ote", bufs=3) as ote,
                tc.tile_pool(name="ps2", bufs=2, space="PSUM") as ps2,
                tc.tile_pool(name="ps3", bufs=3, space="PSUM") as ps3,
            ):
                for c4 in range(NC4):
                    ns = slice(c4 * 512, (c4 + 1) * 512)
                    pouts = [ps2.tile([64, 512], F32, tag=f"pout{h}",
                                      name=f"pout{h}", bufs=1)
                             for h in range(H_LOC)]
                    njc = 4 * (c4 + 1)
                    for h in range(H_LOC):
                        hp = slice(h * 64, (h + 1) * 64)
                        for jc in range(njc):
                            psim = ps3.tile([128, 512], F32, tag="psim")
                            nc.tensor.matmul(
                                psim[:], kt[hp, jc * 128:(jc + 1) * 128],
                                qt[hp, ns], start=True, stop=True)
                            ss = sse.tile([128, 512], F32R, tag="ss")
                            off = jc - 4 * c4
                            if off >= 0:
                                nc.vector.tensor_mul(
                                    ss[:], psim[:],
                                    maskt[:, off * 512:(off + 1) * 512])
                            else:
                                nc.vector.tensor_copy(ss[:], psim[:])
                            nc.tensor.matmul(
                                pouts[h][:],
                                vb[:, jc * 128 + h * 64: jc * 128 + h * 64 + 64],
                                ss[:], start=(jc == 0), stop=(jc == njc - 1))
                    gt_ = gte.tile([128, 512], F32R, tag="gt")
                    for h in range(H_LOC):
                        hp = slice(h * 64, (h + 1) * 64)
                        nc.vector.tensor_mul(gt_[hp, :], pouts[h][:],
                                             gsilu[hp, ns])
                    for ji in range(DCH):
                        poj = ps2.tile([128, 512], F32, tag="poj")
                        nc.tensor.matmul(poj[:], wo_sb[ji][:], gt_[:],
                                         start=True, stop=True)
                        ot = ote.tile([128, 512], F32, tag="ot")
                        nc.scalar.activation(
                            ot[:], poj[:],
                            mybir.ActivationFunctionType.Identity,
                            bias=btile[:, ji:ji + 1])
                        nc.sync.dma_start(OUT[ji * 128:(ji + 1) * 128, ns], ot[:])
    nc.finalize()
    return nc


_NC_CACHE = []


def _get_nc():
    if not _NC_CACHE:
        nc = bacc.Bacc("TRN2", target_bir_lowering=False, debug=False)
        _emit(nc)
        _NC_CACHE.append(nc)
    return _NC_CACHE[0]


def _shard_inputs(x, W_qkv, W_a, W_g, W_out, b_out):
    x2 = np.ascontiguousarray(np.asarray(x, np.float32).reshape(N, D))
    W_qkv = np.asarray(W_qkv, np.float32)
    W_a = np.asarray(W_a, np.float32)
    W_g = np.asarray(W_g, np.float32)
    W_out = np.asarray(W_out, np.float32)
    b_out = np.asarray(b_out, np.float32)

    # W_a column permutation: within a core's 128 cols, source col
    # h*64 + 2d + c  ->  dest col c*64 + h*32 + d
    perm = np.empty(128, np.int64)
    for c in range(2):
        for h in range(2):
            for d in range(DC):
                perm[c * 64 + h * 32 + d] = h * 64 + 2 * d + c

    in_maps = []
    for r in range(NCORES):
        cs = r * 128
        wq = np.ascontiguousarray(W_qkv[:, cs:cs + 128] * np.float32(DH ** -0.5))
        wk = np.ascontiguousarray(W_qkv[:, D + cs:D + cs + 128])
        wv = W_qkv[:, 2 * D + cs:2 * D + cs + 128]
        wa = W_a[:, cs:cs + 128][:, perm]
        wav = np.ascontiguousarray(np.concatenate([wa, wv], axis=1))
        wg = np.ascontiguousarray(W_g[:, cs:cs + 128])
        wo = np.ascontiguousarray(W_out[cs:cs + 128, :])
        if r == 0:
            bt = np.ascontiguousarray(b_out.reshape(8, 128).T)
        else:
            bt = np.zeros((128, 8), np.float32)
        in_maps.append({
            "X": x2, "WQ": wq, "WK": wk, "WAV": wav, "WG": wg, "WO": wo,
            "BT": bt,
        })
    return in_maps


def _unshard(results):
    outT = np.zeros((D, N), np.float32)
    for r in results:
        outT += r["OUT"]
    return np.ascontiguousarray(outT.T).reshape(1, N, D)


def run(trace=False, **inputs):
    nc = _get_nc()
    in_maps = _shard_inputs(**inputs)
    res = run_bass_kernel_spmd(nc, in_maps, core_ids=list(range(NCORES)),
                               trace=trace)
    return _unshard(res.results), res


def kernel(**inputs) -> np.ndarray:
    out, _ = run(trace=False, **inputs)
    return out


# revision 9
# speedup vs baseline: 1.0189x; 1.0189x over previous
"""Trainium2 Bass kernel for nn_CausalFullAttention_37821482009327.

Causal full attention (no softmax) with data-dependent complex relative
position decay, silu gating, and output projection.

Sharding: tensor-parallel over the 16 heads -> 2 heads per NeuronCore x 8.
Each core computes its heads' attention and a partial out-projection
(contraction over its 128-wide dim_inner slice); the host sums the 8
partials (the "all-reduce" happens at gather time).

Shapes (hardcoded): B=1, N=2048, D=1024, H=16, Dh=64, Dc=32.
"""
import sys

sys.path.insert(0, "/opt/trn_rl_repo")

import numpy as np

import concourse.bass as bass
import concourse.tile as tile
from concourse import bacc, mybir
from concourse.bass_utils import run_bass_kernel_spmd
from concourse.masks import make_identity

F32 = mybir.dt.float32
F32R = mybir.dt.float32r  # TF32-class matmul fast path (1 cyc/row vs 4)

N = 2048
D = 1024
H_LOC = 2          # heads per core
DH = 64
DC = 32
NCORES = 8
EPS = 1e-10

NCH = N // 128     # 16 n-chunks of 128
DCH = D // 128     # 8 d-chunks of 128
NC4 = N // 512     # 4 n-chunks of 512


def _emit(nc):
    """Emit the per-core program (SPMD: same program, per-core weight data)."""
    X = nc.dram_tensor("X", [N, D], F32, kind="ExternalInput")
    WQ = nc.dram_tensor("WQ", [D, 128], F32R, kind="ExternalInput")
    WK = nc.dram_tensor("WK", [D, 128], F32R, kind="ExternalInput")
    WA = nc.dram_tensor("WA", [D, 128], F32, kind="ExternalInput")
    WV = nc.dram_tensor("WV", [D, 128], F32R, kind="ExternalInput")
    WG = nc.dram_tensor("WG", [D, 128], F32R, kind="ExternalInput")
    WO = nc.dram_tensor("WO", [128, D], F32R, kind="ExternalInput")
    BT = nc.dram_tensor("BT", [128, 8], F32, kind="ExternalInput")
    OUT = nc.dram_tensor("OUT", [D, N], F32, kind="ExternalOutput")

    with tile.TileContext(nc) as tc:
        with tc.tile_pool(name="pers", bufs=1) as pers:
            # ---- persistent SBUF tensors ----
            ident = pers.tile([128, 128], F32, tag="ident")
            make_identity(nc, ident[:])
            maskt = pers.tile([128, 4 * 512], F32, tag="maskt")
            for off in range(4):
                m = maskt[:, off * 512:(off + 1) * 512]
                nc.gpsimd.memset(m, 1.0)
                # keep (== leave 1.0) where f >= p + 128*off, else 0
                nc.gpsimd.affine_select(
                    out=m, in_=m, compare_op=mybir.AluOpType.is_ge,
                    fill=0.0, base=-128 * off, pattern=[[1, 512]],
                    channel_multiplier=-1)

            xt = [pers.tile([128, N], F32R, tag=f"xt{dc}", name=f"xt{dc}")
                  for dc in range(DCH)]
            qt = pers.tile([128, N], F32R, tag="qt")
            kt = pers.tile([128, N], F32R, tag="kt")
            gsilu = pers.tile([128, N], F32, tag="gsilu")
            acrT = pers.tile([128, N], F32, tag="acrT")
            krT = pers.tile([128, N], F32, tag="krT")
            vb = pers.tile([128, N], F32R, tag="vb")  # v natural [j_lo][nc*128+col]

            # ---- phase A+B: transpose x, fp32 a-projection, scan, acr ----
            with (
                tc.tile_pool(name="xnat", bufs=3) as xnat,
                tc.tile_pool(name="wap", bufs=1) as wap,
                tc.tile_pool(name="scan", bufs=1) as scan,
                tc.tile_pool(name="aep", bufs=3) as aep,
                tc.tile_pool(name="psb", bufs=2, space="PSUM") as psb,
                tc.tile_pool(name="psc", bufs=3, space="PSUM") as psc,
            ):
                wa_sb = [wap.tile([128, 128], F32, tag=f"wa{dc}", name=f"wa{dc}")
                         for dc in range(DCH)]
                for dc in range(DCH):
                    nc.sync.dma_start(wa_sb[dc][:], WA[dc * 128:(dc + 1) * 128, :])

                # scan buffers: [128, 32 blocks-of-64]; block=(nchunk,head),
                # within a block [0:32]=pad, [32:64]=d data
                reA = scan.tile([128, N], F32, tag="reA")
                imA = scan.tile([128, N], F32, tag="imA")
                reB = scan.tile([128, N], F32, tag="reB")
                imB = scan.tile([128, N], F32, tag="imB")
                t1 = scan.tile([128, 1024], F32, tag="t1")
                t2 = scan.tile([128, 1024], F32, tag="t2")

                def blocks(buf, o, w=32):
                    # [p][32 blocks step 64][w cols at offset o]
                    return buf[:].rearrange("p (b w) -> p b w", w=64)[:, :, o:o + w]

                nc.gpsimd.memset(blocks(reA, 0), 1.0)
                nc.gpsimd.memset(blocks(imA, 0), 0.0)
                nc.gpsimd.memset(blocks(reB, 0), 1.0)
                nc.gpsimd.memset(blocks(imB, 0), 0.0)

                for nci in range(NCH):
                    xn = xnat.tile([128, D], F32, tag="xn", name="xn")
                    nc.sync.dma_start(xn[:], X[nci * 128:(nci + 1) * 128, :])
                    pa = psb.tile([128, 128], F32, tag="pa")
                    for dc in range(DCH):
                        pt = psc.tile([128, 128], F32, tag="ptr")
                        nc.tensor.transpose(
                            pt[:], xn[:, dc * 128:(dc + 1) * 128], ident[:])
                        xb = aep.tile([128, 128], F32, tag="xb", name="xb")
                        nc.scalar.copy(xb[:], pt[:])      # exact fp32 xT block
                        nc.tensor.matmul(pa[:], xb[:], wa_sb[dc][:],
                                         start=(dc == 0), stop=(dc == DCH - 1))
                        nc.vector.tensor_copy(             # rounded copy for f32r
                            xt[dc][:, nci * 128:(nci + 1) * 128], xb[:])
                    base = nci * 128
                    dst_re = reA[:, base:base + 128].rearrange(
                        "p (h w) -> p h w", w=64)[:, :, 32:64]
                    dst_im = imA[:, base:base + 128].rearrange(
                        "p (h w) -> p h w", w=64)[:, :, 32:64]
                    src_re = pa[:, 0:64].rearrange("p (h w) -> p h w", w=32)
                    src_im = pa[:, 64:128].rearrange("p (h w) -> p h w", w=32)
                    nc.vector.tensor_copy(dst_re, src_re)
                    nc.vector.tensor_copy(dst_im, src_im)

                # pointwise: ac = a * sigmoid(|a|)/|a|
                t1v = t1[:].rearrange("p (b w) -> p b w", w=32)
                t2v = t2[:].rearrange("p (b w) -> p b w", w=32)
                reD = blocks(reA, 32)
                imD = blocks(imA, 32)
                nc.vector.tensor_mul(t1v, reD, reD)
                nc.vector.tensor_mul(t2v, imD, imD)
                nc.vector.tensor_add(t1v, t1v, t2v)          # |a|^2
                nc.scalar.activation(t2[:], t1[:], mybir.ActivationFunctionType.Sqrt)
                nc.vector.reciprocal_approx_fast(t1[:], t2[:])   # 1/|a|
                nc.scalar.activation(t2[:], t2[:],
                                     mybir.ActivationFunctionType.Sigmoid)
                nc.vector.tensor_mul(t1v, t1v, t2v)          # f = sig(|a|)/|a|
                nc.vector.tensor_mul(reD, reD, t1v)
                nc.vector.tensor_mul(imD, imD, t1v)

                # doubling scan: c[d] = c[d] * c[d - s] (complex), s=1,2,4,8,16
                src_re, src_im, dst_re_b, dst_im_b = reA, imA, reB, imB
                for si, s in enumerate((1, 2, 4, 8, 16)):
                    r0 = blocks(src_re, 32)
                    i0 = blocks(src_im, 32)
                    rs = blocks(src_re, 32 - s)
                    is_ = blocks(src_im, 32 - s)
                    rd = blocks(dst_re_b, 32)
                    id_ = blocks(dst_im_b, 32)
                    nc.vector.tensor_mul(t1v, r0, rs)
                    nc.vector.tensor_mul(t2v, i0, is_)
                    nc.vector.tensor_sub(rd, t1v, t2v)
                    if si < 4:
                        nc.vector.tensor_mul(t1v, r0, is_)
                        nc.vector.tensor_mul(t2v, i0, rs)
                        nc.vector.tensor_add(id_, t1v, t2v)
                    src_re, dst_re_b = dst_re_b, src_re
                    src_im, dst_im_b = dst_im_b, src_im
                # after 5 steps the final real part lives in reB
                for nci in range(NCH):
                    ae = aep.tile([128, 128], F32, tag="ae")
                    base = nci * 128
                    src = reB[:, base:base + 128].rearrange(
                        "p (h w) -> p h w", w=64)[:, :, 32:64]
                    for c in range(2):
                        # dst cols h*64 + 2d + c
                        dst = ae[:].rearrange(
                            "p (h d two) -> p h d two", h=2, two=2)[:, :, :, c]
                        nc.vector.tensor_scalar_max(dst, src, EPS)
                    pae = psc.tile([128, 128], F32, tag="pae")
                    nc.tensor.transpose(pae[:], ae[:], ident[:])
                    nc.vector.tensor_copy(acrT[:, base:base + 128], pae[:])
                nc.vector.reciprocal_approx_fast(krT[:], acrT[:])

            # ---- weights for the remaining phases ----
            with (
                tc.tile_pool(name="wts", bufs=1) as wts,
                tc.tile_pool(name="sse", bufs=4) as sse,
                tc.tile_pool(name="gte", bufs=2) as gte,
                tc.tile_pool(name="ote", bufs=3) as ote,
            ):
                wq_sb = [wts.tile([128, 128], F32R, tag=f"wq{dc}", name=f"wq{dc}")
                         for dc in range(DCH)]
                wk_sb = [wts.tile([128, 128], F32R, tag=f"wk{dc}", name=f"wk{dc}")
                         for dc in range(DCH)]
                wg_sb = [wts.tile([128, 128], F32R, tag=f"wg{dc}", name=f"wg{dc}")
                         for dc in range(DCH)]
                wv_sb = [wts.tile([128, 128], F32R, tag=f"wv{dc}", name=f"wv{dc}")
                         for dc in range(DCH)]
                wo_sb = [wts.tile([128, 128], F32R, tag=f"wo{ji}", name=f"wo{ji}")
                         for ji in range(DCH)]
                btile = wts.tile([128, 8], F32, tag="btile")
                nc.sync.dma_start(btile[:], BT[:])
                for dc in range(DCH):
                    nc.sync.dma_start(wq_sb[dc][:], WQ[dc * 128:(dc + 1) * 128, :])
                    nc.sync.dma_start(wk_sb[dc][:], WK[dc * 128:(dc + 1) * 128, :])
                    nc.sync.dma_start(wg_sb[dc][:], WG[dc * 128:(dc + 1) * 128, :])
                    nc.sync.dma_start(wv_sb[dc][:], WV[dc * 128:(dc + 1) * 128, :])
                    nc.sync.dma_start(wo_sb[dc][:], WO[:, dc * 128:(dc + 1) * 128])

                # ---- phase D: q/k/v/g projections (f32r) ----
                with tc.tile_pool(name="psd", bufs=2, space="PSUM") as psd:
                    for c4 in range(NC4):
                        ns = slice(c4 * 512, (c4 + 1) * 512)
                        pq = psd.tile([128, 512], F32, tag="pq")
                        for dc in range(DCH):
                            nc.tensor.matmul(pq[:], wq_sb[dc][:], xt[dc][:, ns],
                                             start=(dc == 0), stop=(dc == DCH - 1))
                        nc.vector.tensor_mul(qt[:, ns], pq[:], acrT[:, ns])
                        pk = psd.tile([128, 512], F32, tag="pk")
                        for dc in range(DCH):
                            nc.tensor.matmul(pk[:], wk_sb[dc][:], xt[dc][:, ns],
                                             start=(dc == 0), stop=(dc == DCH - 1))
                        nc.vector.tensor_mul(kt[:, ns], pk[:], krT[:, ns])
                        pg = psd.tile([128, 512], F32, tag="pg")
                        for dc in range(DCH):
                            nc.tensor.matmul(pg[:], wg_sb[dc][:], xt[dc][:, ns],
                                             start=(dc == 0), stop=(dc == DCH - 1))
                        nc.scalar.activation(gsilu[:, ns], pg[:],
                                             mybir.ActivationFunctionType.Silu)
                    for nci in range(NCH):
                        pv = psd.tile([128, 128], F32, tag="pv")
                        for dc in range(DCH):
                            nc.tensor.matmul(
                                pv[:],
                                xt[dc][:, nci * 128:(nci + 1) * 128],
                                wv_sb[dc][:], start=(dc == 0),
                                stop=(dc == DCH - 1))
                        nc.vector.tensor_copy(
                            vb[:, nci * 128:(nci + 1) * 128], pv[:])

                # ---- phase E: attention + gating + partial out-projection ----
                with (
                    tc.tile_pool(name="ps2", bufs=2, space="PSUM") as ps2,
                    tc.tile_pool(name="ps3", bufs=3, space="PSUM") as ps3,
                ):
                    for c4 in range(NC4):
                        ns = slice(c4 * 512, (c4 + 1) * 512)
                        pouts = [ps2.tile([64, 512], F32, tag=f"pout{h}",
                                          name=f"pout{h}", bufs=1)
                                 for h in range(H_LOC)]
                        njc = 4 * (c4 + 1)
                        for h in range(H_LOC):
                            hp = slice(h * 64, (h + 1) * 64)
                            for jc in range(njc):
                                psim = ps3.tile([128, 512], F32, tag="psim")
                                nc.tensor.matmul(
                                    psim[:], kt[hp, jc * 128:(jc + 1) * 128],
                                    qt[hp, ns], start=True, stop=True)
                                ss = sse.tile([128, 512], F32R, tag="ss")
                                off = jc - 4 * c4
                                if off >= 0:
                                    nc.vector.tensor_mul(
                                        ss[:], psim[:],
                                        maskt[:, off * 512:(off + 1) * 512])
                                else:
                                    nc.vector.tensor_copy(ss[:], psim[:])
                                nc.tensor.matmul(
                                    pouts[h][:],
                                    vb[:, jc * 128 + h * 64:
                                       jc * 128 + h * 64 + 64],
                                    ss[:], start=(jc == 0),
                                    stop=(jc == njc - 1))
                        gt_ = gte.tile([128, 512], F32R, tag="gt")
                        for h in range(H_LOC):
                            hp = slice(h * 64, (h + 1) * 64)
                            nc.vector.tensor_mul(gt_[hp, :], pouts[h][:],
                                                 gsilu[hp, ns])
                        for ji in range(DCH):
                            poj = ps2.tile([128, 512], F32, tag="poj")
                            nc.tensor.matmul(poj[:], wo_sb[ji][:], gt_[:],
                                             start=True, stop=True)
                            ot = ote.tile([128, 512], F32, tag="ot")
                            nc.scalar.activation(
                                ot[:], poj[:],
                                mybir.ActivationFunctionType.Identity,
                                bias=btile[:, ji:ji + 1])
                            nc.sync.dma_start(
                                OUT[ji * 128:(ji + 1) * 128, ns], ot[:])
    nc.finalize()
    return nc


_NC_CACHE = []


def _get_nc():
    if not _NC_CACHE:
        nc = bacc.Bacc("TRN2", target_bir_lowering=False, debug=False)
        _emit(nc)
        _NC_CACHE.append(nc)
    return _NC_CACHE[0]


def _shard_inputs(x, W_qkv, W_a, W_g, W_out, b_out):
    x2 = np.ascontiguousarray(np.asarray(x, np.float32).reshape(N, D))
    W_qkv = np.asarray(W_qkv, np.float32)
    W_a = np.asarray(W_a, np.float32)
    W_g = np.asarray(W_g, np.float32)
    W_out = np.asarray(W_out, np.float32)
    b_out = np.asarray(b_out, np.float32)

    # W_a column permutation: within a core's 128 cols, source col
    # h*64 + 2d + c  ->  dest col c*64 + h*32 + d
    perm = np.empty(128, np.int64)
    for c in range(2):
        for h in range(2):
            for d in range(DC):
                perm[c * 64 + h * 32 + d] = h * 64 + 2 * d + c

    in_maps = []
    for r in range(NCORES):
        cs = r * 128
        wq = np.ascontiguousarray(W_qkv[:, cs:cs + 128] * np.float32(DH ** -0.5))
        wk = np.ascontiguousarray(W_qkv[:, D + cs:D + cs + 128])
        wv = W_qkv[:, 2 * D + cs:2 * D + cs + 128]
        wa = np.ascontiguousarray(W_a[:, cs:cs + 128][:, perm])
        wv = np.ascontiguousarray(wv)
        wg = np.ascontiguousarray(W_g[:, cs:cs + 128])
        wo = np.ascontiguousarray(W_out[cs:cs + 128, :])
        if r == 0:
            bt = np.ascontiguousarray(b_out.reshape(8, 128).T)
        else:
            bt = np.zeros((128, 8), np.float32)
        in_maps.append({
            "X": x2, "WQ": wq, "WK": wk, "WA": wa, "WV": wv, "WG": wg,
            "WO": wo, "BT": bt,
        })
    return in_maps


def _unshard(results):
    outT = np.zeros((D, N), np.float32)
    for r in results:
        outT += r["OUT"]
    return np.ascontiguousarray(outT.T).reshape(1, N, D)


def run(trace=False, **inputs):
    nc = _get_nc()
    in_maps = _shard_inputs(**inputs)
    res = run_bass_kernel_spmd(nc, in_maps, core_ids=list(range(NCORES)),
                               trace=trace)
    return _unshard(res.results), res


def kernel(**inputs) -> np.ndarray:
    out, _ = run(trace=False, **inputs)
    return out


# revision 16
# speedup vs baseline: 1.0213x; 1.0024x over previous
"""Trainium2 Bass kernel for nn_CausalFullAttention_37821482009327.

Causal full attention (no softmax) with data-dependent complex relative
position decay, silu gating, and output projection.

Sharding: tensor-parallel over the 16 heads -> 2 heads per NeuronCore x 8.
Each core computes its heads' attention and a partial out-projection
(contraction over its 128-wide dim_inner slice); the host sums the 8
partials (the "all-reduce" happens at gather time).

Shapes (hardcoded): B=1, N=2048, D=1024, H=16, Dh=64, Dc=32.
"""
import sys

sys.path.insert(0, "/opt/trn_rl_repo")

import numpy as np

import concourse.bass as bass
import concourse.tile as tile
from concourse import bacc, mybir
from concourse.bass_utils import run_bass_kernel_spmd
from concourse.masks import make_identity

F32 = mybir.dt.float32
F32R = mybir.dt.float32r  # TF32-class matmul fast path (1 cyc/row vs 4)

N = 2048
D = 1024
H_LOC = 2          # heads per core
DH = 64
DC = 32
NCORES = 8
EPS = 1e-10

NCH = N // 128     # 16 n-chunks of 128
DCH = D // 128     # 8 d-chunks of 128
NC4 = N // 512     # 4 n-chunks of 512


def _emit(nc):
    """Emit the per-core program (SPMD: same program, per-core weight data)."""
    X = nc.dram_tensor("X", [N, D], F32, kind="ExternalInput")
    WQ = nc.dram_tensor("WQ", [D, 128], F32R, kind="ExternalInput")
    WK = nc.dram_tensor("WK", [D, 128], F32R, kind="ExternalInput")
    WA = nc.dram_tensor("WA", [D, 128], F32, kind="ExternalInput")
    WV = nc.dram_tensor("WV", [D, 128], F32R, kind="ExternalInput")
    WG = nc.dram_tensor("WG", [D, 128], F32R, kind="ExternalInput")
    WO = nc.dram_tensor("WO", [128, D], F32R, kind="ExternalInput")
    BT = nc.dram_tensor("BT", [128, 8], F32, kind="ExternalInput")
    OUT = nc.dram_tensor("OUT", [D, N], F32, kind="ExternalOutput")

    with tile.TileContext(nc) as tc:
        with tc.tile_pool(name="pers", bufs=1) as pers:
            # ---- persistent SBUF tensors ----
            ident = pers.tile([128, 128], F32, tag="ident")
            make_identity(nc, ident[:])
            xt = [pers.tile([128, N], F32R, tag=f"xt{dc}", name=f"xt{dc}")
                  for dc in range(DCH)]
            qt = pers.tile([128, N], F32R, tag="qt")
            kt = pers.tile([128, N], F32R, tag="kt")
            gsilu = pers.tile([128, N], F32, tag="gsilu")
            acrT = pers.tile([128, N], F32, tag="acrT")
            krT = pers.tile([128, N], F32, tag="krT")
            vb = pers.tile([128, N], F32R, tag="vb")  # v natural [j_lo][nc*128+col]

            # ---- phase A+B: transpose x, fp32 a-projection, scan, acr ----
            with (
                tc.tile_pool(name="xnat", bufs=4) as xnat,
                tc.tile_pool(name="wap", bufs=1) as wap,
                tc.tile_pool(name="scan", bufs=1) as scan,
                tc.tile_pool(name="aep", bufs=3) as aep,
                tc.tile_pool(name="psb", bufs=2, space="PSUM") as psb,
                tc.tile_pool(name="psc", bufs=3, space="PSUM") as psc,
            ):
                wa_sb = [wap.tile([128, 128], F32, tag=f"wa{dc}", name=f"wa{dc}")
                         for dc in range(DCH)]
                for dc in range(DCH):
                    nc.sync.dma_start(wa_sb[dc][:], WA[dc * 128:(dc + 1) * 128, :])

                # scan buffers: [128, 32 blocks-of-64]; block=(nchunk,head),
                # within a block [0:32]=pad, [32:64]=d data
                reA = scan.tile([128, N], F32, tag="reA")
                imA = scan.tile([128, N], F32, tag="imA")
                reB = scan.tile([128, N], F32, tag="reB")
                imB = scan.tile([128, N], F32, tag="imB")
                t1 = scan.tile([128, 1024], F32, tag="t1")
                t2 = scan.tile([128, 1024], F32, tag="t2")

                def blocks(buf, o, w=32):
                    # [p][32 blocks step 64][w cols at offset o]
                    return buf[:].rearrange("p (b w) -> p b w", w=64)[:, :, o:o + w]

                nc.gpsimd.memset(blocks(reA, 0), 1.0)
                nc.gpsimd.memset(blocks(imA, 0), 0.0)
                nc.gpsimd.memset(blocks(reB, 0), 1.0)
                nc.gpsimd.memset(blocks(imB, 0), 0.0)

                for c4 in range(NC4):
                    ns = slice(c4 * 512, (c4 + 1) * 512)
                    xn4 = []
                    for i in range(4):
                        xn = xnat.tile([128, D], F32, tag="xn", name="xn")
                        nci = c4 * 4 + i
                        nc.sync.dma_start(xn[:], X[nci * 128:(nci + 1) * 128, :])
                        xn4.append(xn)
                    xb4 = []
                    for dc in range(DCH):
                        pt = psc.tile([128, 512], F32, tag="ptr")
                        for i in range(4):
                            nc.tensor.transpose(
                                pt[:, i * 128:(i + 1) * 128],
                                xn4[i][:, dc * 128:(dc + 1) * 128], ident[:])
                        xb = aep.tile([128, 512], F32, tag="xb", name="xb",
                                      bufs=4)
                        nc.scalar.copy(xb[:], pt[:])      # exact fp32 xT chunk
                        nc.vector.tensor_copy(xt[dc][:, ns], xb[:])  # f32r copy
                        xb4.append(xb)
                    # aT chunk = wa.T @ xT  (fp32, weights stationary)
                    pa4 = psb.tile([128, 512], F32, tag="pa4")
                    for dc in range(DCH):
                        nc.tensor.matmul(pa4[:], wa_sb[dc][:], xb4[dc][:],
                                         start=(dc == 0), stop=(dc == DCH - 1))
                    at_sb = aep.tile([128, 512], F32, tag="at_sb", name="at_sb", bufs=2)
                    nc.scalar.copy(at_sb[:], pa4[:])
                    # transpose back to natural [n, (c h d)] and drop into the
                    # scan buffers
                    pan = psb.tile([128, 512], F32, tag="pan", bufs=1)
                    for s in range(4):
                        nc.tensor.transpose(
                            pan[:, s * 128:(s + 1) * 128],
                            at_sb[:, s * 128:(s + 1) * 128], ident[:])
                    # pan cols: s*128 + c*64 + h*32 + d ; dst blocks (s,h)
                    dst_re = reA[:, c4 * 512:(c4 + 1) * 512].rearrange(
                        "p (s h w) -> p s h w", s=4, h=2)[:, :, :, 32:64]
                    dst_im = imA[:, c4 * 512:(c4 + 1) * 512].rearrange(
                        "p (s h w) -> p s h w", s=4, h=2)[:, :, :, 32:64]
                    src_re = pan[:].rearrange(
                        "p (s c h d) -> p s c h d", s=4, c=2, h=2)[:, :, 0]
                    src_im = pan[:].rearrange(
                        "p (s c h d) -> p s c h d", s=4, c=2, h=2)[:, :, 1]
                    nc.vector.tensor_copy(dst_re, src_re)
                    nc.vector.tensor_copy(dst_im, src_im)

                # pointwise: ac = a * sigmoid(|a|)/|a|
                t1v = t1[:].rearrange("p (b w) -> p b w", w=32)
                t2v = t2[:].rearrange("p (b w) -> p b w", w=32)
                reD = blocks(reA, 32)
                imD = blocks(imA, 32)
                nc.vector.tensor_mul(t1v, reD, reD)
                nc.vector.tensor_mul(t2v, imD, imD)
                nc.vector.tensor_add(t1v, t1v, t2v)          # |a|^2
                nc.scalar.activation(t2[:], t1[:], mybir.ActivationFunctionType.Sqrt)
                nc.vector.reciprocal_approx_fast(t1[:], t2[:])   # 1/|a|
                nc.scalar.activation(t2[:], t2[:],
                                     mybir.ActivationFunctionType.Sigmoid)
                nc.vector.tensor_mul(t1v, t1v, t2v)          # f = sig(|a|)/|a|
                nc.vector.tensor_mul(reD, reD, t1v)
                nc.vector.tensor_mul(imD, imD, t1v)

                # doubling scan: c[d] = c[d] * c[d - s] (complex), s=1,2,4,8,16
                src_re, src_im, dst_re_b, dst_im_b = reA, imA, reB, imB
                for si, s in enumerate((1, 2, 4, 8, 16)):
                    r0 = blocks(src_re, 32)
                    i0 = blocks(src_im, 32)
                    rs = blocks(src_re, 32 - s)
                    is_ = blocks(src_im, 32 - s)
                    rd = blocks(dst_re_b, 32)
                    id_ = blocks(dst_im_b, 32)
                    nc.vector.tensor_mul(t1v, r0, rs)
                    nc.vector.tensor_mul(t2v, i0, is_)
                    nc.vector.tensor_sub(rd, t1v, t2v)
                    if si < 4:
                        nc.vector.tensor_mul(t1v, r0, is_)
                        nc.vector.tensor_mul(t2v, i0, rs)
                        nc.vector.tensor_add(id_, t1v, t2v)
                    src_re, dst_re_b = dst_re_b, src_re
                    src_im, dst_im_b = dst_im_b, src_im
                # after 5 steps the final real part lives in reB
                for nci in range(NCH):
                    ae = aep.tile([128, 128], F32, tag="ae")
                    base = nci * 128
                    src = reB[:, base:base + 128].rearrange(
                        "p (h w) -> p h w", w=64)[:, :, 32:64]
                    for c in range(2):
                        # dst cols h*64 + 2d + c
                        dst = ae[:].rearrange(
                            "p (h d two) -> p h d two", h=2, two=2)[:, :, :, c]
                        nc.vector.tensor_scalar_max(dst, src, EPS)
                    pae = psb.tile([128, 128], F32, tag="pae", bufs=2)
                    nc.tensor.transpose(pae[:], ae[:], ident[:])
                    nc.vector.tensor_copy(acrT[:, base:base + 128], pae[:])
                nc.vector.reciprocal_approx_fast(krT[:], acrT[:])

            # ---- weights for the remaining phases ----
            with (
                tc.tile_pool(name="wts", bufs=1) as wts,
                tc.tile_pool(name="sse", bufs=4) as sse,
                tc.tile_pool(name="gte", bufs=2) as gte,
                tc.tile_pool(name="ote", bufs=3) as ote,
            ):
                wq_sb = [wts.tile([128, 128], F32R, tag=f"wq{dc}", name=f"wq{dc}")
                         for dc in range(DCH)]
                wk_sb = [wts.tile([128, 128], F32R, tag=f"wk{dc}", name=f"wk{dc}")
                         for dc in range(DCH)]
                wg_sb = [wts.tile([128, 128], F32R, tag=f"wg{dc}", name=f"wg{dc}")
                         for dc in range(DCH)]
                wv_sb = [wts.tile([128, 128], F32R, tag=f"wv{dc}", name=f"wv{dc}")
                         for dc in range(DCH)]
                wo_sb = [wts.tile([128, 128], F32R, tag=f"wo{ji}", name=f"wo{ji}")
                         for ji in range(DCH)]
                maskt = wts.tile([128, 4 * 512], F32, tag="maskt")
                for off in range(4):
                    m = maskt[:, off * 512:(off + 1) * 512]
                    nc.gpsimd.memset(m, 1.0)
                    # keep (== leave 1.0) where f >= p + 128*off, else 0
                    nc.gpsimd.affine_select(
                        out=m, in_=m, compare_op=mybir.AluOpType.is_ge,
                        fill=0.0, base=-128 * off, pattern=[[1, 512]],
                        channel_multiplier=-1)
                btile = wts.tile([128, 8], F32, tag="btile")
                nc.sync.dma_start(btile[:], BT[:])
                for dc in range(DCH):
                    nc.sync.dma_start(wq_sb[dc][:], WQ[dc * 128:(dc + 1) * 128, :])
                    nc.sync.dma_start(wk_sb[dc][:], WK[dc * 128:(dc + 1) * 128, :])
                    nc.sync.dma_start(wg_sb[dc][:], WG[dc * 128:(dc + 1) * 128, :])
                    nc.sync.dma_start(wv_sb[dc][:], WV[dc * 128:(dc + 1) * 128, :])
                    nc.sync.dma_start(wo_sb[dc][:], WO[:, dc * 128:(dc + 1) * 128])

                # ---- phase D: q/k/v/g projections (f32r) ----
                with tc.tile_pool(name="psd", bufs=2, space="PSUM") as psd:
                    for c4 in range(NC4):
                        ns = slice(c4 * 512, (c4 + 1) * 512)
                        pq = psd.tile([128, 512], F32, tag="pq")
                        for dc in range(DCH):
                            nc.tensor.matmul(pq[:], wq_sb[dc][:], xt[dc][:, ns],
                                             start=(dc == 0), stop=(dc == DCH - 1))
                        nc.vector.tensor_mul(qt[:, ns], pq[:], acrT[:, ns])
                        pk = psd.tile([128, 512], F32, tag="pk")
                        for dc in range(DCH):
                            nc.tensor.matmul(pk[:], wk_sb[dc][:], xt[dc][:, ns],
                                             start=(dc == 0), stop=(dc == DCH - 1))
                        nc.vector.tensor_mul(kt[:, ns], pk[:], krT[:, ns])
                        pg = psd.tile([128, 512], F32, tag="pg")
                        for dc in range(DCH):
                            nc.tensor.matmul(pg[:], wg_sb[dc][:], xt[dc][:, ns],
                                             start=(dc == 0), stop=(dc == DCH - 1))
                        nc.scalar.activation(gsilu[:, ns], pg[:],
                                             mybir.ActivationFunctionType.Silu)
                    for c4 in range(NC4):
                        ns = slice(c4 * 512, (c4 + 1) * 512)
                        pv = psd.tile([128, 512], F32, tag="pv", bufs=1)
                        for dc in range(DCH):
                            nc.tensor.matmul(pv[:], wv_sb[dc][:], xt[dc][:, ns],
                                             start=(dc == 0),
                                             stop=(dc == DCH - 1))
                        vt = gte.tile([128, 512], F32, tag="vt", name="vt")
                        nc.vector.tensor_copy(vt[:], pv[:])
                        pvn = psd.tile([128, 512], F32, tag="pvn", bufs=1)
                        for s in range(4):
                            nc.tensor.transpose(
                                pvn[:, s * 128:(s + 1) * 128],
                                vt[:, s * 128:(s + 1) * 128], ident[:])
                        nc.vector.tensor_copy(vb[:, ns], pvn[:])

                # ---- phase E: attention + gating + partial out-projection ----
                with (
                    tc.tile_pool(name="ps2", bufs=2, space="PSUM") as ps2,
                    tc.tile_pool(name="ps3", bufs=3, space="PSUM") as ps3,
                ):
                    for c4 in range(NC4):
                        ns = slice(c4 * 512, (c4 + 1) * 512)
                        pouts = [ps2.tile([64, 512], F32, tag=f"pout{h}",
                                          name=f"pout{h}", bufs=1)
                                 for h in range(H_LOC)]
                        njc = 4 * (c4 + 1)
                        for h in range(H_LOC):
                            hp = slice(h * 64, (h + 1) * 64)
                            for jc in range(njc):
                                psim = ps3.tile([128, 512], F32, tag="psim")
                                nc.tensor.matmul(
                                    psim[:], kt[hp, jc * 128:(jc + 1) * 128],
                                    qt[hp, ns], start=True, stop=True)
                                ss = sse.tile([128, 512], F32R, tag="ss")
                                off = jc - 4 * c4
                                if off >= 0:
                                    nc.vector.tensor_mul(
                                        ss[:], psim[:],
                                        maskt[:, off * 512:(off + 1) * 512])
                                else:
                                    nc.vector.tensor_copy(ss[:], psim[:])
                                nc.tensor.matmul(
                                    pouts[h][:],
                                    vb[:, jc * 128 + h * 64:
                                       jc * 128 + h * 64 + 64],
                                    ss[:], start=(jc == 0),
                                    stop=(jc == njc - 1))
                        gt_ = gte.tile([128, 512], F32R, tag="gt")
                        for h in range(H_LOC):
                            hp = slice(h * 64, (h + 1) * 64)
                            nc.vector.tensor_mul(gt_[hp, :], pouts[h][:],
                                                 gsilu[hp, ns])
                        for ji in range(DCH):
                            poj = ps2.tile([128, 512], F32, tag="poj")
                            nc.tensor.matmul(poj[:], wo_sb[ji][:], gt_[:],
                                             start=True, stop=True)
                            ot = ote.tile([128, 512], F32, tag="ot")
                            nc.scalar.activation(
                                ot[:], poj[:],
                                mybir.ActivationFunctionType.Identity,
                                bias=btile[:, ji:ji + 1])
                            nc.sync.dma_start(
                                OUT[ji * 128:(ji + 1) * 128, ns], ot[:])
    nc.finalize()
    return nc


_NC_CACHE = []


def _get_nc():
    if not _NC_CACHE:
        nc = bacc.Bacc("TRN2", target_bir_lowering=False, debug=False)
        _emit(nc)
        _NC_CACHE.append(nc)
    return _NC_CACHE[0]


def _shard_inputs(x, W_qkv, W_a, W_g, W_out, b_out):
    x2 = np.ascontiguousarray(np.asarray(x, np.float32).reshape(N, D))
    W_qkv = np.asarray(W_qkv, np.float32)
    W_a = np.asarray(W_a, np.float32)
    W_g = np.asarray(W_g, np.float32)
    W_out = np.asarray(W_out, np.float32)
    b_out = np.asarray(b_out, np.float32)

    # W_a column permutation: within a core's 128 cols, source col
    # h*64 + 2d + c  ->  dest col c*64 + h*32 + d
    perm = np.empty(128, np.int64)
    for c in range(2):
        for h in range(2):
            for d in range(DC):
                perm[c * 64 + h * 32 + d] = h * 64 + 2 * d + c

    in_maps = []
    for r in range(NCORES):
        cs = r * 128
        wq = np.ascontiguousarray(W_qkv[:, cs:cs + 128] * np.float32(DH ** -0.5))
        wk = np.ascontiguousarray(W_qkv[:, D + cs:D + cs + 128])
        wv = W_qkv[:, 2 * D + cs:2 * D + cs + 128]
        wa = np.ascontiguousarray(W_a[:, cs:cs + 128][:, perm])
        wv = np.ascontiguousarray(wv)
        wg = np.ascontiguousarray(W_g[:, cs:cs + 128])
        wo = np.ascontiguousarray(W_out[cs:cs + 128, :])
        if r == 0:
            bt = np.ascontiguousarray(b_out.reshape(8, 128).T)
        else:
            bt = np.zeros((128, 8), np.float32)
        in_maps.append({
            "X": x2, "WQ": wq, "WK": wk, "WA": wa, "WV": wv, "WG": wg,
            "WO": wo, "BT": bt,
        })
    return in_maps


def _unshard(results):
    outT = np.zeros((D, N), np.float32)
    for r in results:
        outT += r["OUT"]
    return np.ascontiguousarray(outT.T).reshape(1, N, D)


def run(trace=False, **inputs):
    nc = _get_nc()
    in_maps = _shard_inputs(**inputs)
    res = run_bass_kernel_spmd(nc, in_maps, core_ids=list(range(NCORES)),
                               trace=trace)
    return _unshard(res.results), res


def kernel(**inputs) -> np.ndarray:
    out, _ = run(trace=False, **inputs)
    return out


# revision 18
# speedup vs baseline: 1.0422x; 1.0204x over previous
"""Trainium2 Bass kernel for nn_CausalFullAttention_37821482009327.

Causal full attention (no softmax) with data-dependent complex relative
position decay, silu gating, and output projection.

Sharding: tensor-parallel over the 16 heads -> 2 heads per NeuronCore x 8.
Each core computes its heads' attention and a partial out-projection
(contraction over its 128-wide dim_inner slice); the host sums the 8
partials (the "all-reduce" happens at gather time).

Shapes (hardcoded): B=1, N=2048, D=1024, H=16, Dh=64, Dc=32.
"""
import sys

sys.path.insert(0, "/opt/trn_rl_repo")

import numpy as np

import concourse.bass as bass
import concourse.tile as tile
from concourse import bacc, mybir
from concourse.bass_utils import run_bass_kernel_spmd
from concourse.masks import make_identity

F32 = mybir.dt.float32
F32R = mybir.dt.float32r  # TF32-class matmul fast path (1 cyc/row vs 4)

N = 2048
D = 1024
H_LOC = 2          # heads per core
DH = 64
DC = 32
NCORES = 8
EPS = 1e-10

NCH = N // 128     # 16 n-chunks of 128
DCH = D // 128     # 8 d-chunks of 128
NC4 = N // 512     # 4 n-chunks of 512


def _emit(nc):
    """Emit the per-core program (SPMD: same program, per-core weight data)."""
    X = nc.dram_tensor("X", [N, D], F32, kind="ExternalInput")
    WQ = nc.dram_tensor("WQ", [D, 128], F32R, kind="ExternalInput")
    WK = nc.dram_tensor("WK", [D, 128], F32R, kind="ExternalInput")
    WA = nc.dram_tensor("WA", [D, 128], F32, kind="ExternalInput")
    WV = nc.dram_tensor("WV", [D, 128], F32R, kind="ExternalInput")
    WG = nc.dram_tensor("WG", [D, 128], F32R, kind="ExternalInput")
    WO = nc.dram_tensor("WO", [128, D], F32R, kind="ExternalInput")
    BT = nc.dram_tensor("BT", [128, 8], F32, kind="ExternalInput")
    OUT = nc.dram_tensor("OUT", [D, N], F32, kind="ExternalOutput")

    with tile.TileContext(nc) as tc:
        with tc.tile_pool(name="pers", bufs=1) as pers:
            # ---- persistent SBUF tensors ----
            ident = pers.tile([128, 128], F32, tag="ident")
            make_identity(nc, ident[:])

            xt = [pers.tile([128, N], F32R, tag=f"xt{dc}", name=f"xt{dc}")
                  for dc in range(DCH)]
            qt = pers.tile([128, N], F32R, tag="qt")
            kt = pers.tile([128, N], F32R, tag="kt")
            gsilu = pers.tile([128, N], F32, tag="gsilu")
            acrT = pers.tile([128, N], F32, tag="acrT")
            krT = pers.tile([128, N], F32, tag="krT")
            vb = pers.tile([128, N], F32R, tag="vb")  # v natural [j_lo][nc*128+col]

            # weights: one combined [128, 1024] tile per tensor, slice per
            # 128-chunk. Loaded up-front so projections never wait on SBUF
            # space freed by the scan phase.
            wq_t = pers.tile([128, D], F32R, tag="wq_t")
            wk_t = pers.tile([128, D], F32R, tag="wk_t")
            wg_t = pers.tile([128, D], F32R, tag="wg_t")
            wv_t = pers.tile([128, D], F32R, tag="wv_t")
            wo_t = pers.tile([128, D], F32R, tag="wo_t")
            btile = pers.tile([128, 8], F32, tag="btile")
            for wt, WT in ((wq_t, WQ), (wk_t, WK), (wg_t, WG), (wv_t, WV)):
                nc.sync.dma_start(
                    wt[:].rearrange("p (dc c) -> p dc c", dc=DCH),
                    WT[:].rearrange("(dc p) c -> p dc c", p=128))
            nc.sync.dma_start(wo_t[:], WO[:])
            nc.sync.dma_start(btile[:], BT[:])
            wq_sb = [wq_t[:, dc * 128:(dc + 1) * 128] for dc in range(DCH)]
            wk_sb = [wk_t[:, dc * 128:(dc + 1) * 128] for dc in range(DCH)]
            wg_sb = [wg_t[:, dc * 128:(dc + 1) * 128] for dc in range(DCH)]
            wv_sb = [wv_t[:, dc * 128:(dc + 1) * 128] for dc in range(DCH)]
            wo_sb = [wo_t[:, ji * 128:(ji + 1) * 128] for ji in range(DCH)]

            # ---- phase A+B: transpose x, fp32 a-projection, scan, acr ----
            with (
                tc.tile_pool(name="xnat", bufs=4) as xnat,
                tc.tile_pool(name="wap", bufs=1) as wap,
                tc.tile_pool(name="scan", bufs=1) as scan,
                tc.tile_pool(name="aep", bufs=3) as aep,
                tc.tile_pool(name="psb", bufs=2, space="PSUM") as psb,
                tc.tile_pool(name="psc", bufs=3, space="PSUM") as psc,
            ):
                wa_sb = [wap.tile([128, 128], F32, tag=f"wa{dc}", name=f"wa{dc}")
                         for dc in range(DCH)]
                for dc in range(DCH):
                    nc.sync.dma_start(wa_sb[dc][:], WA[dc * 128:(dc + 1) * 128, :])

                # scan buffers: [128, 1024] = [nchunk 16][head 2][d 32]
                reA = scan.tile([128, 1024], F32, tag="reA")
                imA = scan.tile([128, 1024], F32, tag="imA")
                reB = scan.tile([128, 1024], F32, tag="reB")
                imB = scan.tile([128, 1024], F32, tag="imB")
                t1 = scan.tile([128, 1024], F32, tag="t1")
                t2 = scan.tile([128, 1024], F32, tag="t2")

                def blk(buf, lo, hi):
                    # [p][32 blocks step 32][cols lo:hi]
                    return buf[:].rearrange(
                        "p (b w) -> p b w", w=32)[:, :, lo:hi]

                for c4 in range(NC4):
                    ns = slice(c4 * 512, (c4 + 1) * 512)
                    xn4 = []
                    for i in range(4):
                        xn = xnat.tile([128, D], F32, tag="xn", name="xn")
                        nci = c4 * 4 + i
                        nc.sync.dma_start(xn[:], X[nci * 128:(nci + 1) * 128, :])
                        xn4.append(xn)
                    xb4 = []
                    for dc in range(DCH):
                        pt = psc.tile([128, 512], F32, tag="ptr")
                        for i in range(4):
                            nc.tensor.transpose(
                                pt[:, i * 128:(i + 1) * 128],
                                xn4[i][:, dc * 128:(dc + 1) * 128], ident[:])
                        xb = aep.tile([128, 512], F32, tag="xb", name="xb",
                                      bufs=5)
                        nc.scalar.copy(xb[:], pt[:])      # exact fp32 xT chunk
                        nc.vector.tensor_copy(xt[dc][:, ns], xb[:])  # f32r copy
                        xb4.append(xb)
                    # aT chunk = wa.T @ xT  (fp32, weights stationary)
                    pa4 = psb.tile([128, 512], F32, tag="pa4")
                    for dc in range(DCH):
                        nc.tensor.matmul(pa4[:], wa_sb[dc][:], xb4[dc][:],
                                         start=(dc == 0), stop=(dc == DCH - 1))
                    at_sb = aep.tile([128, 512], F32, tag="xb", name="at_sb",
                                     bufs=5)
                    nc.scalar.copy(at_sb[:], pa4[:])
                    # transpose back to natural [n, (c h d)] into scan buffers
                    pan = psb.tile([128, 512], F32, tag="pan", bufs=1)
                    for s in range(4):
                        nc.tensor.transpose(
                            pan[:, s * 128:(s + 1) * 128],
                            at_sb[:, s * 128:(s + 1) * 128], ident[:])
                    # pan cols: s*128 + c*64 + h*32 + d ; dst blocks (s,h)
                    dst_re = reA[:, c4 * 256:(c4 + 1) * 256].rearrange(
                        "p (s h d) -> p s h d", s=4, h=2)
                    dst_im = imA[:, c4 * 256:(c4 + 1) * 256].rearrange(
                        "p (s h d) -> p s h d", s=4, h=2)
                    src_re = pan[:].rearrange(
                        "p (s c h d) -> p s c h d", s=4, c=2, h=2)[:, :, 0]
                    src_im = pan[:].rearrange(
                        "p (s c h d) -> p s c h d", s=4, c=2, h=2)[:, :, 1]
                    nc.vector.tensor_copy(dst_re, src_re)
                    nc.vector.tensor_copy(dst_im, src_im)

                # pointwise: ac = a * sigmoid(|a|)/|a|   (contiguous [128,1024])
                nc.vector.tensor_mul(t1[:], reA[:], reA[:])
                nc.vector.tensor_mul(t2[:], imA[:], imA[:])
                nc.vector.tensor_add(t1[:], t1[:], t2[:])          # |a|^2
                nc.scalar.activation(t2[:], t1[:], mybir.ActivationFunctionType.Sqrt)
                nc.vector.reciprocal_approx_fast(t1[:], t2[:])     # 1/|a|
                nc.scalar.activation(t2[:], t2[:],
                                     mybir.ActivationFunctionType.Sigmoid)
                nc.vector.tensor_mul(t1[:], t1[:], t2[:])          # sig(|a|)/|a|
                nc.vector.tensor_mul(reA[:], reA[:], t1[:])
                nc.vector.tensor_mul(imA[:], imA[:], t1[:])

                # doubling scan: c[d] = c[d] * c[d - s] (complex), s=1,2,4,8,16
                # prefix d < s copies through unchanged.
                src_re_b, src_im_b, dst_re_b, dst_im_b = reA, imA, reB, imB
                for si, s in enumerate((1, 2, 4, 8, 16)):
                    w = 32 - s
                    r0 = blk(src_re_b, s, 32)
                    i0 = blk(src_im_b, s, 32)
                    rs = blk(src_re_b, 0, w)
                    is_ = blk(src_im_b, 0, w)
                    rd = blk(dst_re_b, s, 32)
                    id_ = blk(dst_im_b, s, 32)
                    tt1 = blk(t1, 0, w)
                    tt2 = blk(t2, 0, w)
                    nc.vector.tensor_copy(blk(dst_re_b, 0, s), blk(src_re_b, 0, s))
                    nc.vector.tensor_mul(tt1, r0, rs)
                    nc.vector.tensor_mul(tt2, i0, is_)
                    nc.vector.tensor_sub(rd, tt1, tt2)
                    if si < 4:
                        nc.vector.tensor_copy(blk(dst_im_b, 0, s),
                                              blk(src_im_b, 0, s))
                        nc.vector.tensor_mul(tt1, r0, is_)
                        nc.vector.tensor_mul(tt2, i0, rs)
                        nc.vector.tensor_add(id_, tt1, tt2)
                    src_re_b, dst_re_b = dst_re_b, src_re_b
                    src_im_b, dst_im_b = dst_im_b, src_im_b
                # after 5 steps the final real part lives in reB
                for nci in range(NCH):
                    ae = aep.tile([128, 128], F32, tag="ae", bufs=2)
                    src = reB[:, nci * 64:(nci + 1) * 64].rearrange(
                        "p (h d) -> p h d", h=2)
                    for c in range(2):
                        # dst cols h*64 + 2d + c
                        dst = ae[:].rearrange(
                            "p (h d two) -> p h d two", h=2, two=2)[:, :, :, c]
                        nc.vector.tensor_scalar_max(dst, src, EPS)
                    pae = psb.tile([128, 128], F32, tag="pae", bufs=2)
                    nc.tensor.transpose(pae[:], ae[:], ident[:])
                    nc.vector.tensor_copy(acrT[:, nci * 128:(nci + 1) * 128],
                                          pae[:])
                nc.vector.reciprocal_approx_fast(krT[:], acrT[:])

            # ---- phase D: v/g then q/k projections (f32r) ----
            with (
                tc.tile_pool(name="psd", bufs=2, space="PSUM") as psd,
                tc.tile_pool(name="vgp", bufs=2) as vgp,
            ):
                for c4 in range(NC4):
                    ns = slice(c4 * 512, (c4 + 1) * 512)
                    pv = psd.tile([128, 512], F32, tag="pv", bufs=1)
                    for dc in range(DCH):
                        nc.tensor.matmul(pv[:], wv_sb[dc], xt[dc][:, ns],
                                         start=(dc == 0), stop=(dc == DCH - 1))
                    vtile = vgp.tile([128, 512], F32, tag="vt", name="vt")
                    nc.vector.tensor_copy(vtile[:], pv[:])
                    pvn = psd.tile([128, 512], F32, tag="pvn", bufs=1)
                    for s in range(4):
                        nc.tensor.transpose(
                            pvn[:, s * 128:(s + 1) * 128],
                            vtile[:, s * 128:(s + 1) * 128], ident[:])
                    nc.vector.tensor_copy(vb[:, ns], pvn[:])
                    pg = psd.tile([128, 512], F32, tag="pg")
                    for dc in range(DCH):
                        nc.tensor.matmul(pg[:], wg_sb[dc], xt[dc][:, ns],
                                         start=(dc == 0), stop=(dc == DCH - 1))
                    nc.scalar.activation(gsilu[:, ns], pg[:],
                                         mybir.ActivationFunctionType.Silu)
                for c4 in range(NC4):
                    ns = slice(c4 * 512, (c4 + 1) * 512)
                    pq = psd.tile([128, 512], F32, tag="pq")
                    for dc in range(DCH):
                        nc.tensor.matmul(pq[:], wq_sb[dc], xt[dc][:, ns],
                                         start=(dc == 0), stop=(dc == DCH - 1))
                    nc.vector.tensor_mul(qt[:, ns], pq[:], acrT[:, ns])
                    pk = psd.tile([128, 512], F32, tag="pk")
                    for dc in range(DCH):
                        nc.tensor.matmul(pk[:], wk_sb[dc], xt[dc][:, ns],
                                         start=(dc == 0), stop=(dc == DCH - 1))
                    nc.vector.tensor_mul(kt[:, ns], pk[:], krT[:, ns])

            # ---- phase E: attention + gating + partial out-projection ----
            with (
                tc.tile_pool(name="sse", bufs=4) as sse,
                tc.tile_pool(name="gte", bufs=2) as gte,
                tc.tile_pool(name="ote", bufs=3) as ote,
                tc.tile_pool(name="ps2", bufs=2, space="PSUM") as ps2,
                tc.tile_pool(name="ps3", bufs=3, space="PSUM") as ps3,
            ):
                maskt = gte.tile([128, 4 * 512], F32, tag="maskt", bufs=1)
                for off in range(4):
                    m = maskt[:, off * 512:(off + 1) * 512]
                    nc.gpsimd.memset(m, 1.0)
                    # keep (== leave 1.0) where f >= p + 128*off, else 0
                    nc.gpsimd.affine_select(
                        out=m, in_=m, compare_op=mybir.AluOpType.is_ge,
                        fill=0.0, base=-128 * off, pattern=[[1, 512]],
                        channel_multiplier=-1)
                for c4 in range(NC4):
                    ns = slice(c4 * 512, (c4 + 1) * 512)
                    pouts = [ps2.tile([64, 512], F32, tag=f"pout{h}",
                                      name=f"pout{h}", bufs=1)
                             for h in range(H_LOC)]
                    njc = 4 * (c4 + 1)
                    for h in range(H_LOC):
                        hp = slice(h * 64, (h + 1) * 64)
                        for jc in range(njc):
                            psim = ps3.tile([128, 512], F32, tag="psim")
                            nc.tensor.matmul(
                                psim[:], kt[hp, jc * 128:(jc + 1) * 128],
                                qt[hp, ns], start=True, stop=True)
                            ss = sse.tile([128, 512], F32R, tag="ss")
                            off = jc - 4 * c4
                            if off >= 0:
                                nc.vector.tensor_mul(
                                    ss[:], psim[:],
                                    maskt[:, off * 512:(off + 1) * 512])
                            else:
                                nc.vector.tensor_copy(ss[:], psim[:])
                            nc.tensor.matmul(
                                pouts[h][:],
                                vb[:, jc * 128 + h * 64:
                                   jc * 128 + h * 64 + 64],
                                ss[:], start=(jc == 0),
                                stop=(jc == njc - 1))
                    gt_ = gte.tile([128, 512], F32R, tag="gt")
                    for h in range(H_LOC):
                        hp = slice(h * 64, (h + 1) * 64)
                        nc.vector.tensor_mul(gt_[hp, :], pouts[h][:],
                                             gsilu[hp, ns])
                    for ji in range(DCH):
                        poj = ps2.tile([128, 512], F32, tag="poj")
                        nc.tensor.matmul(poj[:], wo_sb[ji], gt_[:],
                                         start=True, stop=True)
                        ot = ote.tile([128, 512], F32, tag="ot")
                        nc.scalar.activation(
                            ot[:], poj[:],
                            mybir.ActivationFunctionType.Identity,
                            bias=btile[:, ji:ji + 1])
                        nc.sync.dma_start(
                            OUT[ji * 128:(ji + 1) * 128, ns], ot[:])
    nc.finalize()
    return nc


_NC_CACHE = []


def _get_nc():
    if not _NC_CACHE:
        nc = bacc.Bacc("TRN2", target_bir_lowering=False, debug=False)
        _emit(nc)
        _NC_CACHE.append(nc)
    return _NC_CACHE[0]


def _shard_inputs(x, W_qkv, W_a, W_g, W_out, b_out):
    x2 = np.ascontiguousarray(np.asarray(x, np.float32).reshape(N, D))
    W_qkv = np.asarray(W_qkv, np.float32)
    W_a = np.asarray(W_a, np.float32)
    W_g = np.asarray(W_g, np.float32)
    W_out = np.asarray(W_out, np.float32)
    b_out = np.asarray(b_out, np.float32)

    # W_a column permutation: within a core's 128 cols, source col
    # h*64 + 2d + c  ->  dest col c*64 + h*32 + d
    perm = np.empty(128, np.int64)
    for c in range(2):
        for h in range(2):
            for d in range(DC):
                perm[c * 64 + h * 32 + d] = h * 64 + 2 * d + c

    in_maps = []
    for r in range(NCORES):
        cs = r * 128
        wq = np.ascontiguousarray(W_qkv[:, cs:cs + 128] * np.float32(DH ** -0.5))
        wk = np.ascontiguousarray(W_qkv[:, D + cs:D + cs + 128])
        wv = W_qkv[:, 2 * D + cs:2 * D + cs + 128]
        wa = np.ascontiguousarray(W_a[:, cs:cs + 128][:, perm])
        wv = np.ascontiguousarray(wv)
        wg = np.ascontiguousarray(W_g[:, cs:cs + 128])
        wo = np.ascontiguousarray(W_out[cs:cs + 128, :])
        if r == 0:
            bt = np.ascontiguousarray(b_out.reshape(8, 128).T)
        else:
            bt = np.zeros((128, 8), np.float32)
        in_maps.append({
            "X": x2, "WQ": wq, "WK": wk, "WA": wa, "WV": wv, "WG": wg,
            "WO": wo, "BT": bt,
        })
    return in_maps


def _unshard(results):
    outT = np.zeros((D, N), np.float32)
    for r in results:
        outT += r["OUT"]
    return np.ascontiguousarray(outT.T).reshape(1, N, D)


def run(trace=False, **inputs):
    nc = _get_nc()
    in_maps = _shard_inputs(**inputs)
    res = run_bass_kernel_spmd(nc, in_maps, core_ids=list(range(NCORES)),
                               trace=trace)
    return _unshard(res.results), res


def kernel(**inputs) -> np.ndarray:
    out, _ = run(trace=False, **inputs)
    return out


# revision 19
# speedup vs baseline: 1.0485x; 1.0061x over previous
"""Trainium2 Bass kernel for nn_CausalFullAttention_37821482009327.

Causal full attention (no softmax) with data-dependent complex relative
position decay, silu gating, and output projection.

Sharding: tensor-parallel over the 16 heads -> 2 heads per NeuronCore x 8.
Each core computes its heads' attention and a partial out-projection
(contraction over its 128-wide dim_inner slice); the host sums the 8
partials (the "all-reduce" happens at gather time).

Shapes (hardcoded): B=1, N=2048, D=1024, H=16, Dh=64, Dc=32.
"""
import sys

sys.path.insert(0, "/opt/trn_rl_repo")

import numpy as np

import concourse.bass as bass
import concourse.tile as tile
from concourse import bacc, mybir
from concourse.bass_utils import run_bass_kernel_spmd
from concourse.masks import make_identity

F32 = mybir.dt.float32
F32R = mybir.dt.float32r  # TF32-class matmul fast path (1 cyc/row vs 4)

N = 2048
D = 1024
H_LOC = 2          # heads per core
DH = 64
DC = 32
NCORES = 8
EPS = 1e-10

NCH = N // 128     # 16 n-chunks of 128
DCH = D // 128     # 8 d-chunks of 128
NC4 = N // 512     # 4 n-chunks of 512


def _emit(nc):
    """Emit the per-core program (SPMD: same program, per-core weight data)."""
    X = nc.dram_tensor("X", [N, D], F32, kind="ExternalInput")
    WQ = nc.dram_tensor("WQ", [D, 128], F32R, kind="ExternalInput")
    WK = nc.dram_tensor("WK", [D, 128], F32R, kind="ExternalInput")
    WA = nc.dram_tensor("WA", [D, 128], F32, kind="ExternalInput")
    WV = nc.dram_tensor("WV", [D, 128], F32R, kind="ExternalInput")
    WG = nc.dram_tensor("WG", [D, 128], F32R, kind="ExternalInput")
    WO = nc.dram_tensor("WO", [128, D], F32R, kind="ExternalInput")
    BT = nc.dram_tensor("BT", [128, 8], F32, kind="ExternalInput")
    OUT = nc.dram_tensor("OUT", [D, N], F32, kind="ExternalOutput")

    with tile.TileContext(nc) as tc:
        with (
            tc.tile_pool(name="pers", bufs=1) as pers,
            tc.tile_pool(name="ps", bufs=2, space="PSUM") as ps,
        ):
            # ---- persistent SBUF tensors ----
            ident = pers.tile([128, 128], F32, tag="ident")
            make_identity(nc, ident[:])

            xt = [pers.tile([128, N], F32R, tag=f"xt{dc}", name=f"xt{dc}")
                  for dc in range(DCH)]
            qt = pers.tile([128, N], F32R, tag="qt")
            kt = pers.tile([128, N], F32R, tag="kt")
            gsilu = pers.tile([128, N], F32, tag="gsilu")
            acrT = pers.tile([128, N], F32, tag="acrT")
            krT = pers.tile([128, N], F32, tag="krT")
            vb = pers.tile([128, N], F32R, tag="vb")  # v natural [j_lo][nc*128+col]

            # weights: one combined [128, 1024] tile per tensor, slice per
            # 128-chunk. Loaded up-front so projections never wait on SBUF
            # space freed by the scan phase.
            wq_t = pers.tile([128, D], F32R, tag="wq_t")
            wk_t = pers.tile([128, D], F32R, tag="wk_t")
            wg_t = pers.tile([128, D], F32R, tag="wg_t")
            wv_t = pers.tile([128, D], F32R, tag="wv_t")
            wo_t = pers.tile([128, D], F32R, tag="wo_t")
            btile = pers.tile([128, 8], F32, tag="btile")
            for wt, WT in ((wq_t, WQ), (wk_t, WK), (wg_t, WG), (wv_t, WV)):
                nc.sync.dma_start(
                    wt[:].rearrange("p (dc c) -> p dc c", dc=DCH),
                    WT[:].rearrange("(dc p) c -> p dc c", p=128))
            nc.sync.dma_start(wo_t[:], WO[:])
            nc.sync.dma_start(btile[:], BT[:])
            wq_sb = [wq_t[:, dc * 128:(dc + 1) * 128] for dc in range(DCH)]
            wk_sb = [wk_t[:, dc * 128:(dc + 1) * 128] for dc in range(DCH)]
            wg_sb = [wg_t[:, dc * 128:(dc + 1) * 128] for dc in range(DCH)]
            wv_sb = [wv_t[:, dc * 128:(dc + 1) * 128] for dc in range(DCH)]
            wo_sb = [wo_t[:, ji * 128:(ji + 1) * 128] for ji in range(DCH)]

            # ---- phase A+B: transpose x, fp32 a-projection, scan, acr ----
            with (
                tc.tile_pool(name="xnat", bufs=4) as xnat,
                tc.tile_pool(name="wap", bufs=1) as wap,
                tc.tile_pool(name="scan", bufs=1) as scan,
                tc.tile_pool(name="aep", bufs=3) as aep,
            ):
                wa_sb = [wap.tile([128, 128], F32, tag=f"wa{dc}", name=f"wa{dc}")
                         for dc in range(DCH)]
                for dc in range(DCH):
                    nc.sync.dma_start(wa_sb[dc][:], WA[dc * 128:(dc + 1) * 128, :])

                # scan buffers: [128, 1024] = [nchunk 16][head 2][d 32]
                reA = scan.tile([128, 1024], F32, tag="reA")
                imA = scan.tile([128, 1024], F32, tag="imA")
                reB = scan.tile([128, 1024], F32, tag="reB")
                imB = scan.tile([128, 1024], F32, tag="imB")
                t1 = scan.tile([128, 1024], F32, tag="t1")
                t2 = scan.tile([128, 1024], F32, tag="t2")

                def blk(buf, lo, hi):
                    # [p][32 blocks step 32][cols lo:hi]
                    return buf[:].rearrange(
                        "p (b w) -> p b w", w=32)[:, :, lo:hi]

                for c4 in range(NC4):
                    ns = slice(c4 * 512, (c4 + 1) * 512)
                    xn4 = []
                    for i in range(4):
                        xn = xnat.tile([128, D], F32, tag="xn", name="xn")
                        nci = c4 * 4 + i
                        nc.sync.dma_start(xn[:], X[nci * 128:(nci + 1) * 128, :])
                        xn4.append(xn)
                    xb4 = []
                    for dc in range(DCH):
                        pt = ps.tile([128, 512], F32, tag="ptr")
                        for i in range(4):
                            nc.tensor.transpose(
                                pt[:, i * 128:(i + 1) * 128],
                                xn4[i][:, dc * 128:(dc + 1) * 128], ident[:])
                        xb = aep.tile([128, 512], F32, tag="xb", name="xb",
                                      bufs=5)
                        nc.scalar.copy(xb[:], pt[:])      # exact fp32 xT chunk
                        nc.vector.tensor_copy(xt[dc][:, ns], xb[:])  # f32r copy
                        xb4.append(xb)
                    # aT chunk = wa.T @ xT  (fp32, weights stationary)
                    pa4 = ps.tile([128, 512], F32, tag="acc")
                    for dc in range(DCH):
                        nc.tensor.matmul(pa4[:], wa_sb[dc][:], xb4[dc][:],
                                         start=(dc == 0), stop=(dc == DCH - 1))
                    at_sb = aep.tile([128, 512], F32, tag="xb", name="at_sb",
                                     bufs=5)
                    nc.scalar.copy(at_sb[:], pa4[:])
                    # transpose back to natural [n, (c h d)] into scan buffers
                    pan = ps.tile([128, 512], F32, tag="ptr")
                    for s in range(4):
                        nc.tensor.transpose(
                            pan[:, s * 128:(s + 1) * 128],
                            at_sb[:, s * 128:(s + 1) * 128], ident[:])
                    # pan cols: s*128 + c*64 + h*32 + d ; dst blocks (s,h)
                    dst_re = reA[:, c4 * 256:(c4 + 1) * 256].rearrange(
                        "p (s h d) -> p s h d", s=4, h=2)
                    dst_im = imA[:, c4 * 256:(c4 + 1) * 256].rearrange(
                        "p (s h d) -> p s h d", s=4, h=2)
                    src_re = pan[:].rearrange(
                        "p (s c h d) -> p s c h d", s=4, c=2, h=2)[:, :, 0]
                    src_im = pan[:].rearrange(
                        "p (s c h d) -> p s c h d", s=4, c=2, h=2)[:, :, 1]
                    nc.vector.tensor_copy(dst_re, src_re)
                    nc.vector.tensor_copy(dst_im, src_im)

                # pointwise: ac = a * sigmoid(|a|)/|a|   (contiguous [128,1024])
                nc.vector.tensor_mul(t1[:], reA[:], reA[:])
                nc.vector.tensor_mul(t2[:], imA[:], imA[:])
                nc.vector.tensor_add(t1[:], t1[:], t2[:])          # |a|^2
                nc.scalar.activation(t2[:], t1[:], mybir.ActivationFunctionType.Sqrt)
                nc.vector.reciprocal_approx_fast(t1[:], t2[:])     # 1/|a|
                nc.scalar.activation(t2[:], t2[:],
                                     mybir.ActivationFunctionType.Sigmoid)
                nc.vector.tensor_mul(t1[:], t1[:], t2[:])          # sig(|a|)/|a|
                nc.vector.tensor_mul(reA[:], reA[:], t1[:])
                nc.vector.tensor_mul(imA[:], imA[:], t1[:])

                # doubling scan: c[d] = c[d] * c[d - s] (complex), s=1,2,4,8,16
                # prefix d < s copies through unchanged.
                src_re_b, src_im_b, dst_re_b, dst_im_b = reA, imA, reB, imB
                for si, s in enumerate((1, 2, 4, 8, 16)):
                    w = 32 - s
                    r0 = blk(src_re_b, s, 32)
                    i0 = blk(src_im_b, s, 32)
                    rs = blk(src_re_b, 0, w)
                    is_ = blk(src_im_b, 0, w)
                    rd = blk(dst_re_b, s, 32)
                    id_ = blk(dst_im_b, s, 32)
                    tt1 = blk(t1, 0, w)
                    tt2 = blk(t2, 0, w)
                    nc.vector.tensor_copy(blk(dst_re_b, 0, s), blk(src_re_b, 0, s))
                    nc.vector.tensor_mul(tt1, r0, rs)
                    nc.vector.tensor_mul(tt2, i0, is_)
                    nc.vector.tensor_sub(rd, tt1, tt2)
                    if si < 4:
                        nc.vector.tensor_copy(blk(dst_im_b, 0, s),
                                              blk(src_im_b, 0, s))
                        nc.vector.tensor_mul(tt1, r0, is_)
                        nc.vector.tensor_mul(tt2, i0, rs)
                        nc.vector.tensor_add(id_, tt1, tt2)
                    src_re_b, dst_re_b = dst_re_b, src_re_b
                    src_im_b, dst_im_b = dst_im_b, src_im_b
                # after 5 steps the final real part lives in reB
                for nci in range(NCH):
                    ae = aep.tile([128, 128], F32, tag="ae", bufs=2)
                    src = reB[:, nci * 64:(nci + 1) * 64].rearrange(
                        "p (h d) -> p h d", h=2)
                    for c in range(2):
                        # dst cols h*64 + 2d + c
                        dst = ae[:].rearrange(
                            "p (h d two) -> p h d two", h=2, two=2)[:, :, :, c]
                        nc.vector.tensor_scalar_max(dst, src, EPS)
                    pae = ps.tile([128, 128], F32, tag="ptr")
                    nc.tensor.transpose(pae[:], ae[:], ident[:])
                    nc.vector.tensor_copy(acrT[:, nci * 128:(nci + 1) * 128],
                                          pae[:])
                nc.vector.reciprocal_approx_fast(krT[:], acrT[:])

            # ---- phase D: v/g then q/k projections (f32r) ----
            with tc.tile_pool(name="vgp", bufs=2) as vgp:
                for c4 in range(NC4):
                    ns = slice(c4 * 512, (c4 + 1) * 512)
                    pv = ps.tile([128, 512], F32, tag="acc")
                    for dc in range(DCH):
                        nc.tensor.matmul(pv[:], wv_sb[dc], xt[dc][:, ns],
                                         start=(dc == 0), stop=(dc == DCH - 1))
                    vtile = vgp.tile([128, 512], F32, tag="vt", name="vt")
                    nc.vector.tensor_copy(vtile[:], pv[:])
                    pvn = ps.tile([128, 512], F32, tag="ptr")
                    for s in range(4):
                        nc.tensor.transpose(
                            pvn[:, s * 128:(s + 1) * 128],
                            vtile[:, s * 128:(s + 1) * 128], ident[:])
                    nc.vector.tensor_copy(vb[:, ns], pvn[:])
                    pg = ps.tile([128, 512], F32, tag="acc")
                    for dc in range(DCH):
                        nc.tensor.matmul(pg[:], wg_sb[dc], xt[dc][:, ns],
                                         start=(dc == 0), stop=(dc == DCH - 1))
                    nc.scalar.activation(gsilu[:, ns], pg[:],
                                         mybir.ActivationFunctionType.Silu)
                for c4 in range(NC4):
                    ns = slice(c4 * 512, (c4 + 1) * 512)
                    pq = ps.tile([128, 512], F32, tag="acc")
                    for dc in range(DCH):
                        nc.tensor.matmul(pq[:], wq_sb[dc], xt[dc][:, ns],
                                         start=(dc == 0), stop=(dc == DCH - 1))
                    nc.vector.tensor_mul(qt[:, ns], pq[:], acrT[:, ns])
                    pk = ps.tile([128, 512], F32, tag="acc")
                    for dc in range(DCH):
                        nc.tensor.matmul(pk[:], wk_sb[dc], xt[dc][:, ns],
                                         start=(dc == 0), stop=(dc == DCH - 1))
                    nc.vector.tensor_mul(kt[:, ns], pk[:], krT[:, ns])

            # ---- phase E: attention + gating + partial out-projection ----
            with (
                tc.tile_pool(name="sse", bufs=4) as sse,
                tc.tile_pool(name="gte", bufs=2) as gte,
                tc.tile_pool(name="ote", bufs=3) as ote,
            ):
                maskt = gte.tile([128, 4 * 512], F32, tag="maskt", bufs=1)
                for off in range(4):
                    m = maskt[:, off * 512:(off + 1) * 512]
                    nc.gpsimd.memset(m, 1.0)
                    # keep (== leave 1.0) where f >= p + 128*off, else 0
                    nc.gpsimd.affine_select(
                        out=m, in_=m, compare_op=mybir.AluOpType.is_ge,
                        fill=0.0, base=-128 * off, pattern=[[1, 512]],
                        channel_multiplier=-1)
                for c4 in range(NC4):
                    ns = slice(c4 * 512, (c4 + 1) * 512)
                    pouts = [ps.tile([64, 512], F32, tag=f"pout{h}",
                                     name=f"pout{h}", bufs=1)
                             for h in range(H_LOC)]
                    njc = 4 * (c4 + 1)
                    for h in range(H_LOC):
                        hp = slice(h * 64, (h + 1) * 64)
                        for jc in range(njc):
                            psim = ps.tile([128, 512], F32, tag="psim")
                            nc.tensor.matmul(
                                psim[:], kt[hp, jc * 128:(jc + 1) * 128],
                                qt[hp, ns], start=True, stop=True)
                            ss = sse.tile([128, 512], F32R, tag="ss")
                            off = jc - 4 * c4
                            if off >= 0:
                                nc.vector.tensor_mul(
                                    ss[:], psim[:],
                                    maskt[:, off * 512:(off + 1) * 512])
                            else:
                                nc.scalar.copy(ss[:], psim[:])
                            nc.tensor.matmul(
                                pouts[h][:],
                                vb[:, jc * 128 + h * 64:
                                   jc * 128 + h * 64 + 64],
                                ss[:], start=(jc == 0),
                                stop=(jc == njc - 1))
                    gt_ = gte.tile([128, 512], F32R, tag="gt")
                    for h in range(H_LOC):
                        hp = slice(h * 64, (h + 1) * 64)
                        nc.vector.tensor_mul(gt_[hp, :], pouts[h][:],
                                             gsilu[hp, ns])
                    for ji in range(DCH):
                        poj = ps.tile([128, 512], F32, tag="acc")
                        nc.tensor.matmul(poj[:], wo_sb[ji], gt_[:],
                                         start=True, stop=True)
                        ot = ote.tile([128, 512], F32, tag="ot")
                        nc.scalar.activation(
                            ot[:], poj[:],
                            mybir.ActivationFunctionType.Identity,
                            bias=btile[:, ji:ji + 1])
                        nc.sync.dma_start(
                            OUT[ji * 128:(ji + 1) * 128, ns], ot[:])
    nc.finalize()
    return nc


_NC_CACHE = []


def _get_nc():
    if not _NC_CACHE:
        nc = bacc.Bacc("TRN2", target_bir_lowering=False, debug=False)
        _emit(nc)
        _NC_CACHE.append(nc)
    return _NC_CACHE[0]


def _shard_inputs(x, W_qkv, W_a, W_g, W_out, b_out):
    x2 = np.ascontiguousarray(np.asarray(x, np.float32).reshape(N, D))
    W_qkv = np.asarray(W_qkv, np.float32)
    W_a = np.asarray(W_a, np.float32)
    W_g = np.asarray(W_g, np.float32)
    W_out = np.asarray(W_out, np.float32)
    b_out = np.asarray(b_out, np.float32)

    # W_a column permutation: within a core's 128 cols, source col
    # h*64 + 2d + c  ->  dest col c*64 + h*32 + d
    perm = np.empty(128, np.int64)
    for c in range(2):
        for h in range(2):
            for d in range(DC):
                perm[c * 64 + h * 32 + d] = h * 64 + 2 * d + c

    in_maps = []
    for r in range(NCORES):
        cs = r * 128
        wq = np.ascontiguousarray(W_qkv[:, cs:cs + 128] * np.float32(DH ** -0.5))
        wk = np.ascontiguousarray(W_qkv[:, D + cs:D + cs + 128])
        wv = W_qkv[:, 2 * D + cs:2 * D + cs + 128]
        wa = np.ascontiguousarray(W_a[:, cs:cs + 128][:, perm])
        wv = np.ascontiguousarray(wv)
        wg = np.ascontiguousarray(W_g[:, cs:cs + 128])
        wo = np.ascontiguousarray(W_out[cs:cs + 128, :])
        if r == 0:
            bt = np.ascontiguousarray(b_out.reshape(8, 128).T)
        else:
            bt = np.zeros((128, 8), np.float32)
        in_maps.append({
            "X": x2, "WQ": wq, "WK": wk, "WA": wa, "WV": wv, "WG": wg,
            "WO": wo, "BT": bt,
        })
    return in_maps


def _unshard(results):
    outT = np.zeros((D, N), np.float32)
    for r in results:
        outT += r["OUT"]
    return np.ascontiguousarray(outT.T).reshape(1, N, D)


def run(trace=False, **inputs):
    nc = _get_nc()
    in_maps = _shard_inputs(**inputs)
    res = run_bass_kernel_spmd(nc, in_maps, core_ids=list(range(NCORES)),
                               trace=trace)
    return _unshard(res.results), res


def kernel(**inputs) -> np.ndarray:
    out, _ = run(trace=False, **inputs)
    return out
